# revision 1
# baseline (speedup 1.0000x reference)
"""ChannelCovarianceBlock Trainium2 kernel.

Computes, for queries x1 (B, C, h, w) and support sets x2 (nw, Bs, C, h, w):
  cov_n = Cov(x2[n].reshape(Bs*C, hw))            (hw, hw) per class
  d     = normalize-and-center rows of x1.reshape(B*C, hw)
  sim[b, n, c] = d[bc] @ cov_n @ d[bc]^T          -> (B, nw*C)

Sharding: data-parallel over B across 8 NeuronCores (32 queries each);
each core computes all 10 class covariances from the full x2 (redundant
but collective-free) using the Gram identity cov = (X^T X - s s^T/N)/(N-1).

Per-core dataflow:
  stage 0: preprocess queries in place (SBUF-resident D), build D^T via
           PE transposes, spill D^T to DRAM.
  stage 1 (per class): Gram matmuls + rank-1 mean correction -> cov_n SBUF.
  stage 2 (per class, m-tile): S = D @ cov_n on PE (float32r), then a
           fused multiply+row-reduce (scalar_tensor_tensor with accum_out;
           tensor_tensor_reduce crashes trn2 HW) against the resident D
           gives sim[:, n].

Measured (8 cores, steady state): ~1.6-1.9 ms/exec (device-state
dependent), rel err 4.5e-05.
"""

import os
import sys

for _p in ("/opt/trn_rl_repo", "/root/.axon_site/_ro/trn_rl_repo"):
    if os.path.isdir(_p) and _p not in sys.path:
        sys.path.append(_p)

import numpy as np

# Problem constants (hardcoded per spec).
B, C, H, W = 256, 128, 28, 28
NW, BS = 10, 10
HW = H * W            # 784
N_CORES = 8
BSH = B // N_CORES    # 32 queries per core
NI = BSH * C          # 4096 rows per core
NR = BS * C           # 1280 support rows per class

# K-tiles over the hw contraction dim (partition dim <= 128).
KT = [(k * 128, min(128, HW - k * 128)) for k in range((HW + 127) // 128)]
NKT = len(KT)         # 7
# N-tiles over the hw free dim (>=256 keeps float32r at 1 cycle/row).
QT = [(0, 392), (392, 392)]
MT = NI // 128        # 32 i-tiles per core

_STATE = {}


def _build_program(mm_dtype_name="float32r", stages=None, repeat=None):
    if stages is None:
        stages = os.environ.get("CCB_STAGES", "full")
    if repeat is None:
        repeat = int(os.environ.get("CCB_REPEAT", "1"))
    import concourse.bass as bass
    import concourse.bacc as bacc
    import concourse.tile as tile
    from concourse import mybir
    from concourse.masks import make_identity
    from contextlib import ExitStack

    f32 = mybir.dt.float32
    # Matmul operand dtype: float32r runs the PE at 1 cycle/row (vs 4 for
    # fp32) for N>=256. All f32r-consumed tiles must be f32r-typed with
    # walrus-approved producers (DMA from f32r DRAM, or DVE/ACT rounding
    # copies) -- the BIR verifier enforces this.
    mmdt = getattr(mybir.dt, mm_dtype_name)

    nc = bacc.Bacc()
    x1s = nc.declare_dram_parameter("x1s", [NI, HW], f32, isOutput=False)
    x2d = nc.declare_dram_parameter("x2", [NW, NR, HW], mmdt, isOutput=False)
    out = nc.declare_dram_parameter("out", [NI, NW], f32, isOutput=True)

    AF = mybir.ActivationFunctionType
    OP = mybir.AluOpType

    with tile.TileContext(nc) as tc:
        with ExitStack() as ctx:
            persist = ctx.enter_context(tc.tile_pool(name="persist", bufs=1))
            ident = persist.tile([128, 128], f32, tag="ident")
            make_identity(nc, ident)
            ones_f = persist.tile([128, 1], f32, tag="ones_f")
            nc.vector.memset(ones_f, 1.0)
            ones = persist.tile([128, 1], mmdt, tag="ones")
            nc.vector.tensor_copy(out=ones, in_=ones_f)
            # D stays resident: d_res[:, m, q] = D[m*128 + p, q]
            d_res = persist.tile([128, MT, HW], f32, tag="d_res")
            out_acc = persist.tile([128, MT, NW], f32, tag="out_acc")
            if stages != "full":
                nc.vector.memset(out_acc, 0.0)

            dram = ctx.enter_context(tc.tile_pool(name="dram", bufs=1, space="DRAM"))
            # dtT_dram[m][p, kt, i] = D[m*128 + i, kt*128 + p] (full k-blocks)
            dtT_dram = dram.tile([MT, 128, NKT - 1, 128], mmdt, tag="dtT")
            # remainder k-block (16 rows of p)
            dtr_dram = dram.tile([MT, KT[-1][1], 128], mmdt, tag="dtr")

            scr_pool = ctx.enter_context(tc.tile_pool(name="scr", bufs=2))
            stats = ctx.enter_context(tc.tile_pool(name="stats", bufs=4))

            # Optional on-device repeat loop (timing amplification only).
            if repeat > 1:
                ctx.enter_context(tc.For_i(0, repeat, 1))

            # ---- Stage 0: query preprocessing + D^T build ----
            with tc.tile_pool(name="psum_t", bufs=2, space="PSUM") as psum_t, \
                 tc.tile_pool(name="dtw", bufs=2) as dtw_pool:
                for m in range(MT):
                    rows = slice(m * 128, (m + 1) * 128)
                    dsl = d_res[:, m, :]
                    nc.sync.dma_start(out=dsl, in_=x1s[rows, :])
                    sq = scr_pool.tile([128, HW], f32, tag="scr")
                    sumsq = stats.tile([128, 1], f32, tag="sumsq")
                    # ACT: sq = x^2 (discarded), sumsq = row-sum(x^2)
                    nc.scalar.activation(
                        out=sq, in_=dsl, func=AF.Square, accum_out=sumsq
                    )
                    s1 = stats.tile([128, 1], f32, tag="s1")
                    nc.vector.tensor_reduce(
                        out=s1, in_=dsl, axis=mybir.AxisListType.X, op=OP.add
                    )
                    nrm = stats.tile([128, 1], f32, tag="nrm")
                    nc.scalar.activation(out=nrm, in_=sumsq, func=AF.Sqrt)
                    rn = stats.tile([128, 1], f32, tag="rn")
                    nc.vector.reciprocal(out=rn, in_=nrm)
                    ms = stats.tile([128, 1], f32, tag="ms")
                    nc.scalar.mul(out=ms, in_=s1, mul=1.0 / HW)
                    # d = (x - mean) * (1/||x||), in place
                    nc.vector.tensor_scalar(
                        out=dsl, in0=dsl, scalar1=ms, scalar2=rn,
                        op0=OP.subtract, op1=OP.mult,
                    )
                    dtw = dtw_pool.tile([128, NKT - 1, 128], mmdt, tag="dtw")
                    dtr = dtw_pool.tile([KT[-1][1], 128], mmdt, tag="dtr")
                    for kt, (koff, klen) in enumerate(KT):
                        pt = psum_t.tile([128, 128], f32, tag="pt")
                        nc.tensor.transpose(
                            out=pt[:klen, :128],
                            in_=dsl[:, koff:koff + klen],
                            identity=ident,
                        )
                        dst = dtw[:, kt, :] if kt < NKT - 1 else dtr
                        nc.vector.tensor_copy(out=dst, in_=pt[:klen, :128])
                    nc.sync.dma_start(out=dtT_dram[m], in_=dtw)
                    nc.sync.dma_start(out=dtr_dram[m], in_=dtr)

            # ---- Stages 1+2 per class ----
            xs_pool = ctx.enter_context(tc.tile_pool(name="xsup", bufs=1))
            cov_pool = ctx.enter_context(tc.tile_pool(name="cov", bufs=2))
            row_pool = ctx.enter_context(tc.tile_pool(name="rows", bufs=1))
            dts_pool = ctx.enter_context(tc.tile_pool(name="dts", bufs=4))
            psum_s = ctx.enter_context(
                tc.tile_pool(name="psum_s", bufs=4, space="PSUM")
            )
            psum_m = ctx.enter_context(
                tc.tile_pool(name="psum_mean", bufs=1, space="PSUM")
            )

            RTN = NR // 128  # 10 row-tiles per class
            for n in range(NW if stages != "0" else 0):
                xs = xs_pool.tile([128, RTN, HW], mmdt, tag="xs")
                for rt in range(RTN):
                    nc.sync.dma_start(
                        out=xs[:, rt, :], in_=x2d[n, rt * 128:(rt + 1) * 128, :]
                    )
                # column sums s (1, HW) via ones-matmul; psum sub-tiles are
                # bank-aligned (512-elem stride) so no matmul crosses a bank.
                pm = psum_m.tile([1, len(QT), 512], f32, tag="pm")
                for rt in range(RTN):
                    for qi, (qoff, qlen) in enumerate(QT):
                        nc.tensor.matmul(
                            pm[:1, qi, :qlen],
                            lhsT=ones[:, :1],
                            rhs=xs[:, rt, qoff:qoff + qlen],
                            start=(rt == 0),
                            stop=(rt == RTN - 1),
                        )
                srow = row_pool.tile([1, HW], mmdt, tag="srow")
                ssrow = row_pool.tile([1, HW], mmdt, tag="ssrow")
                for qi, (qoff, qlen) in enumerate(QT):
                    qs = slice(qoff, qoff + qlen)
                    nc.scalar.mul(out=srow[:, qs], in_=pm[:1, qi, :qlen], mul=1.0)
                    nc.scalar.mul(out=ssrow[:, qs], in_=pm[:1, qi, :qlen], mul=-1.0 / NR)

                # cov_n = (X^T X - s s^T / NR) / (NR - 1), tiled (p-block, q)
                cov = cov_pool.tile([128, NKT, HW], mmdt, tag="cov")
                for mc, (mcoff, mclen) in enumerate(KT):
                    for (qoff, qlen) in QT:
                        ps = psum_s.tile([128, 392], f32, tag="ps")
                        for rt in range(RTN):
                            nc.tensor.matmul(
                                ps[:mclen, :qlen],
                                lhsT=xs[:, rt, mcoff:mcoff + mclen],
                                rhs=xs[:, rt, qoff:qoff + qlen],
                                start=(rt == 0),
                                stop=False,
                            )
                        nc.tensor.matmul(
                            ps[:mclen, :qlen],
                            lhsT=ssrow[:1, mcoff:mcoff + mclen],
                            rhs=srow[:1, qoff:qoff + qlen],
                            start=False,
                            stop=True,
                        )
                        nc.scalar.mul(
                            out=cov[:mclen, mc, qoff:qoff + qlen],
                            in_=ps[:mclen, :qlen],
                            mul=1.0 / (NR - 1),
                        )

                # Stage 2: sim[:, n] = rowsum((D @ cov_n) * D) per m-tile
                for m in range(MT if stages not in ("0", "01") else 0):
                    dtw2 = dts_pool.tile([128, NKT - 1, 128], mmdt, tag="dts")
                    # 3-way split: parallel HWDGE queues for the hot D^T
                    # stream (1 big DMA = 1 queue = too slow; 6-way = too
                    # many dispatches). Measured optimum.
                    nc.sync.dma_start(out=dtw2[:, :2, :], in_=dtT_dram[m, :, :2, :])
                    nc.sync.dma_start(out=dtw2[:, 2:4, :], in_=dtT_dram[m, :, 2:4, :])
                    nc.sync.dma_start(out=dtw2[:, 4:, :], in_=dtT_dram[m, :, 4:, :])
                    dtr2 = dts_pool.tile([KT[-1][1], 128], mmdt, tag="dtsr")
                    nc.sync.dma_start(out=dtr2, in_=dtr_dram[m])
                    if stages == "2d":
                        scr = scr_pool.tile([128, HW], f32, tag="scr")
                        nc.vector.tensor_copy(out=scr[:, :128], in_=dtw2[:, 0, :])
                        continue
                    if stages == "2w":
                        # matmuls with weights from a fixed resident tile
                        # (no dependence on the streamed dtw2) - stall probe
                        for qi, (qoff, qlen) in enumerate(QT):
                            ps = psum_s.tile([128, 392], f32, name="ps", tag="ps")
                            for kt, (koff, klen) in enumerate(KT):
                                nc.tensor.matmul(
                                    ps[:128, :qlen],
                                    lhsT=xs[:klen, 0, :128],
                                    rhs=cov[:klen, kt, qoff:qoff + qlen],
                                    start=(kt == 0),
                                    stop=(kt == NKT - 1),
                                )
                            scr = scr_pool.tile([128, HW], f32, tag="scr")
                            nc.vector.tensor_copy(out=scr[:, :qlen], in_=ps[:, :qlen])
                        continue
                    acc = out_acc[:, m, n:n + 1]
                    pp = stats.tile([128, 2], f32, name="pp", tag="pp")
                    for qi, (qoff, qlen) in enumerate(QT):
                        ps = psum_s.tile([128, 392], f32, tag="ps")
                        for kt, (koff, klen) in enumerate(KT):
                            lhsT = dtw2[:, kt, :] if kt < NKT - 1 else dtr2
                            nc.tensor.matmul(
                                ps[:128, :qlen],
                                lhsT=lhsT,
                                rhs=cov[:klen, kt, qoff:qoff + qlen],
                                start=(kt == 0),
                                stop=(kt == NKT - 1),
                            )
                        scr = scr_pool.tile([128, HW], f32, tag="scr")
                        # out=(ps*1)*d elementwise; accum_out = row-sum
                        nc.vector.scalar_tensor_tensor(
                            out=scr[:, :qlen],
                            in0=ps[:, :qlen],
                            scalar=1.0,
                            in1=d_res[:, m, qoff:qoff + qlen],
                            op0=OP.mult,
                            op1=OP.mult,
                            accum_out=pp[:, qi:qi + 1],
                        )
                    nc.vector.tensor_reduce(
                        out=acc, in_=pp, axis=mybir.AxisListType.X, op=OP.add
                    )

            for m in range(MT):
                nc.sync.dma_start(
                    out=out[m * 128:(m + 1) * 128, :], in_=out_acc[:, m, :]
                )

    # Bacc defers register allocation to compile(); run_bass_via_pjrt
    # serializes the module as-is, so finalize here.
    nc.finalize()
    return nc


def get_program():
    key = "nc"
    if key not in _STATE:
        _STATE[key] = _build_program(
            os.environ.get("CCB_MM_DTYPE", "float32r")
        )
    return _STATE[key]


def make_in_maps(x1, x2):
    x1f = np.ascontiguousarray(
        np.asarray(x1, dtype=np.float32).reshape(B * C, HW)
    )
    x2f = np.ascontiguousarray(
        np.asarray(x2, dtype=np.float32).reshape(NW, NR, HW)
    )
    return [
        {"x1s": x1f[c * NI:(c + 1) * NI], "x2": x2f}
        for c in range(N_CORES)
    ]


def assemble_output(core_outs):
    # per-core (NI, NW) -> (BSH, NW*C); concat over cores -> (B, NW*C)
    parts = [
        o.reshape(BSH, C, NW).transpose(0, 2, 1).reshape(BSH, NW * C)
        for o in core_outs
    ]
    return np.ascontiguousarray(np.concatenate(parts, axis=0), dtype=np.float32)


def kernel(x1, x2):
    from concourse.bass_utils import run_bass_kernel_spmd

    nc = get_program()
    in_maps = make_in_maps(x1, x2)
    res = run_bass_kernel_spmd(nc, in_maps, list(range(N_CORES)))
    return assemble_output([res.results[i]["out"] for i in range(N_CORES)])



# revision 6
# speedup vs baseline: 1.8141x; 1.8141x over previous
"""ChannelCovarianceBlock Trainium2 kernel (v2: fp8 DoubleRow, all-resident).

Computes, for queries x1 (B, C, h, w) and support sets x2 (nw, Bs, C, h, w):
  cov_n = Cov(x2[n].reshape(Bs*C, hw))            (hw, hw) per class
  d     = normalize-and-center rows of x1.reshape(B*C, hw)
  sim[b, n, c] = d[bc] @ cov_n @ d[bc]^T          -> (B, nw*C)

Sharding: data-parallel over B across 8 NeuronCores (32 queries each);
each core computes all 10 class covariances from the full x2 (redundant
but collective-free) using the Gram identity cov = (X^T X - s s^T/N)/(N-1).

v2 design vs v1 (fp32r, D^T restreamed from DRAM 10x = 128MB/core):
  * Everything SBUF-resident: cov for all 10 classes (fp8), D^T (fp8),
    D (bf16). Zero DRAM traffic after the initial input loads.
  * fp8e4 DoubleRow matmuls (2 MACs/cell/cycle) for the Gram stage and
    the dominant S = D @ cov stage.
  * fp8 precision rescue: raw fp8 fails the 2e-2 gate (measured 2.1e-2)
    because cov ~ I and the diagonal ~1.0 quantizes at 6%. Fix: subtract
    SHIFT*I inside the PSUM accumulation (one bf16 identity matmul per
    cov tile), so fp8 stores cov' = cov - 1.0008*I with entries ~ +-0.05,
    then add back 1.0008*||d||^2 (computed exactly in f32) at the end.
    Power-of-2 pre-scales (x2*16, d*16, cov*64) dodge fp8 subnormals.
"""

import os
import sys

for _p in ("/opt/trn_rl_repo", "/root/.axon_site/_ro/trn_rl_repo"):
    if os.path.isdir(_p) and _p not in sys.path:
        sys.path.append(_p)

import numpy as np

# Problem constants (hardcoded per spec).
B, C, H, W = 256, 128, 28, 28
NW, BS = 10, 10
HW = H * W            # 784
N_CORES = 8
BSH = B // N_CORES    # 32 queries per core
NI = BSH * C          # 4096 rows per core
NR = BS * C           # 1280 support rows per class

# K-tiles over the hw contraction dim (partition dim <= 128).
KT = [(k * 128, min(128, HW - k * 128)) for k in range((HW + 127) // 128)]
NKT = len(KT)         # 7 (6 full 128-tiles + one 16-row remainder)
NDR = 3               # DoubleRow pairs covering k-tiles 0..5
QT = [(0, 392), (392, 392)]
MT = NI // 128        # 32 i-tiles per core
RTN = NR // 128       # 10 support row-tiles per class

# fp8 scaling scheme (all powers of two; see module docstring).
XSC = 16.0                            # host-side x2 scale
DSC = 16.0                            # on-device d scale
CSC = 64.0                            # cov fp8 scale
SHIFT_PSUM = 327680.0                 # 5*2^16, bf16-exact; ~= XSC^2*(NR-1)
ADD_BACK = SHIFT_PSUM / (XSC * XSC * (NR - 1))   # 1.000782...
COV_MUL = CSC / (XSC * XSC * (NR - 1))           # PSUM -> cov8 scale
STT_MUL = 1.0 / (DSC * DSC * CSC)                # PSUM*d_res -> sim units
SSD_MUL = ADD_BACK / (DSC * DSC)                 # ACT-square accum -> add-back

_STATE = {}


def _build_program(mm_dtype_name=None, stages=None, repeat=None,
                   nw_count=NW, mt_count=MT):
    if repeat is None:
        repeat = int(os.environ.get("CCB_REPEAT", "1"))
    import concourse.bass as bass
    import concourse.bacc as bacc
    import concourse.tile as tile
    from concourse import mybir
    from concourse.masks import make_identity
    from contextlib import ExitStack

    f32 = mybir.dt.float32
    bf16 = mybir.dt.bfloat16
    f8 = mybir.dt.float8e4
    DR = mybir.MatmulPerfMode.DoubleRow

    nc = bacc.Bacc()
    x1s = nc.declare_dram_parameter("x1s", [NI, HW], f32, isOutput=False)
    x2d = nc.declare_dram_parameter("x2", [NW, NR, HW], f8, isOutput=False)
    out = nc.declare_dram_parameter("out", [NI, NW], f32, isOutput=True)

    AF = mybir.ActivationFunctionType
    OP = mybir.AluOpType

    with tile.TileContext(nc) as tc:
        with ExitStack() as ctx:
            persist = ctx.enter_context(tc.tile_pool(name="persist", bufs=1))
            ident = persist.tile([128, 128], f32, tag="ident")
            make_identity(nc, ident)
            ident_bf = persist.tile([128, 128], bf16, tag="ident_bf")
            nc.vector.tensor_copy(out=ident_bf, in_=ident)
            # pair-dim stride must be 16B-aligned for DoubleRow ldweights,
            # so pad the ones tile to [128, 2, 16] and slice [:, :, :1].
            ones_f = persist.tile([128, 2, 16], f32, tag="ones_f")
            nc.vector.memset(ones_f, 1.0)
            ones8 = persist.tile([128, 2, 16], f8, tag="ones8")
            nc.vector.tensor_copy(out=ones8, in_=ones_f)
            # sid[k, mc, q] = -SHIFT_PSUM * delta(q == mc*128 + k): the
            # rank-128 identity block used to shift cov's diagonal inside
            # the PSUM accumulation.
            sid = persist.tile([128, NKT, HW], bf16, tag="sid")
            nc.vector.memset(sid, 0.0)
            for mc, (mo, ml) in enumerate(KT):
                nc.scalar.mul(out=sid[:, mc, mo:mo + ml],
                              in_=ident[:, :ml], mul=-SHIFT_PSUM)
            # D resident both ways: rows for the final elementwise reduce,
            # transposed fp8 (scaled 16x) as stage-2 matmul weights.
            d_res = persist.tile([128, MT, HW], bf16, tag="d_res")
            dtT = persist.tile([128, MT, NKT, 128], f8, tag="dtT")
            # cov' fp8 (scaled 64x) for all classes, k-tile-major rows.
            cov8 = persist.tile([128, NW, NKT, HW], f8, tag="cov8")
            ssd = persist.tile([128, MT], f32, tag="ssd")
            out_acc = persist.tile([128, MT, NW], f32, tag="out_acc")
            if nw_count < NW or mt_count < MT:
                nc.vector.memset(out_acc, 0.0)  # reduced (sim-only) builds

            xw_pool = ctx.enter_context(tc.tile_pool(name="xw", bufs=2))
            dc_pool = ctx.enter_context(tc.tile_pool(name="dc", bufs=2))
            stats = ctx.enter_context(tc.tile_pool(name="stats", bufs=6))
            scr_pool = ctx.enter_context(tc.tile_pool(name="scr", bufs=2))
            s8_pool = ctx.enter_context(tc.tile_pool(name="scr8", bufs=2))
            xs_pool = ctx.enter_context(tc.tile_pool(name="xsup", bufs=2))
            row_pool = ctx.enter_context(tc.tile_pool(name="rows", bufs=2))

            # Optional on-device repeat loop (timing amplification only).
            if repeat > 1:
                ctx.enter_context(tc.For_i(0, repeat, 1))

            # ---- Stage 0: query preprocessing, d (bf16) + 16*d^T (fp8) ----
            with tc.tile_pool(name="psum_t", bufs=2, space="PSUM") as psum_t:
                for m in range(mt_count):
                    rows = slice(m * 128, (m + 1) * 128)
                    xw = xw_pool.tile([128, HW], f32, tag="xw")
                    nc.sync.dma_start(out=xw, in_=x1s[rows, :])
                    sq = scr_pool.tile([128, HW], f32, tag="scr")
                    sumsq = stats.tile([128, 1], f32, tag="sumsq")
                    # ACT: sq = x^2 (discarded), sumsq = row-sum(x^2)
                    nc.scalar.activation(
                        out=sq, in_=xw, func=AF.Square, accum_out=sumsq
                    )
                    s1 = stats.tile([128, 1], f32, tag="s1")
                    nc.vector.tensor_reduce(
                        out=s1, in_=xw, axis=mybir.AxisListType.X, op=OP.add
                    )
                    # nrm16 = ||x|| / 16, rn = 16 / ||x||
                    nrm16 = stats.tile([128, 1], f32, tag="nrm")
                    nc.scalar.activation(out=nrm16, in_=sumsq, func=AF.Sqrt,
                                         scale=1.0 / (DSC * DSC))
                    rn = stats.tile([128, 1], f32, tag="rn")
                    nc.vector.reciprocal(out=rn, in_=nrm16)
                    ms = stats.tile([128, 1], f32, tag="ms")
                    nc.scalar.mul(out=ms, in_=s1, mul=1.0 / HW)
                    # dc = (x - mean) * (16/||x||) = 16 * d
                    dc = dc_pool.tile([128, HW], f32, tag="dc")
                    nc.vector.tensor_scalar(
                        out=dc, in0=xw, scalar1=ms, scalar2=rn,
                        op0=OP.subtract, op1=OP.mult,
                    )
                    nc.vector.tensor_copy(out=d_res[:, m, :], in_=dc)
                    # ssd[:, m] = 1.0008 * ||d||^2 (from 256*||d||^2 accum)
                    sq2 = scr_pool.tile([128, HW], f32, tag="scr")
                    ssd2 = stats.tile([128, 1], f32, tag="ssd2")
                    nc.scalar.activation(
                        out=sq2, in_=dc, func=AF.Square, accum_out=ssd2
                    )
                    nc.scalar.mul(out=ssd[:, m:m + 1], in_=ssd2, mul=SSD_MUL)
                    for kt, (ko, kl) in enumerate(KT):
                        pt = psum_t.tile([128, 128], f32, tag="pt")
                        nc.tensor.transpose(
                            out=pt[:kl, :128], in_=dc[:, ko:ko + kl],
                            identity=ident,
                        )
                        nc.vector.tensor_copy(out=dtT[:kl, m, kt, :],
                                              in_=pt[:kl, :128])

            # ---- Stage 1 (per class): cov' = cov - 1.0008*I in fp8 ----
            psum_m = ctx.enter_context(
                tc.tile_pool(name="psum_m", bufs=1, space="PSUM"))
            psum_s = ctx.enter_context(
                tc.tile_pool(name="psum_s", bufs=4, space="PSUM"))
            for n in range(nw_count):
                xs = xs_pool.tile([128, RTN, HW], f8, tag="xs")
                for rt in range(RTN):
                    nc.sync.dma_start(
                        out=xs[:, rt, :],
                        in_=x2d[n, rt * 128:(rt + 1) * 128, :])
                # column sums of 16*x via fp8 DoubleRow ones-matmul
                pm = psum_m.tile([1, 2, 512], f32, tag="pm")
                for j in range(RTN // 2):
                    for qi, (qo, ql) in enumerate(QT):
                        nc.tensor.matmul(
                            pm[:1, qi, :ql], lhsT=ones8[:, :, :1],
                            rhs=xs[:, 2 * j:2 * j + 2, qo:qo + ql],
                            start=(j == 0), stop=(j == RTN // 2 - 1),
                            perf_mode=DR,
                        )
                srow = row_pool.tile([1, HW], bf16, tag="srow")
                ssrow = row_pool.tile([1, HW], bf16, tag="ssrow")
                for qi, (qo, ql) in enumerate(QT):
                    nc.scalar.mul(out=srow[:, qo:qo + ql],
                                  in_=pm[:1, qi, :ql], mul=1.0)
                    nc.scalar.mul(out=ssrow[:, qo:qo + ql],
                                  in_=pm[:1, qi, :ql], mul=-1.0 / NR)
                for mc, (mo, ml) in enumerate(KT):
                    ps2 = [psum_s.tile([128, 392], f32, name="ps", tag="ps")
                           for _ in QT]
                    for j in range(RTN // 2):
                        for qi, (qo, ql) in enumerate(QT):
                            nc.tensor.matmul(
                                ps2[qi][:ml, :ql],
                                lhsT=xs[:, 2 * j:2 * j + 2, mo:mo + ml],
                                rhs=xs[:, 2 * j:2 * j + 2, qo:qo + ql],
                                start=(j == 0), stop=False, perf_mode=DR,
                            )
                    for qi, (qo, ql) in enumerate(QT):
                        # diagonal shift (only where block mc overlaps qh)
                        if mo < qo + ql and mo + ml > qo:
                            nc.tensor.matmul(
                                ps2[qi][:ml, :ql], lhsT=ident_bf[:, :ml],
                                rhs=sid[:, mc, qo:qo + ql],
                                start=False, stop=False,
                            )
                        # rank-1 mean correction: -= s s^T / NR
                        nc.tensor.matmul(
                            ps2[qi][:ml, :ql], lhsT=ssrow[:1, mo:mo + ml],
                            rhs=srow[:1, qo:qo + ql],
                            start=False, stop=True,
                        )
                        nc.scalar.mul(out=cov8[:ml, n, mc, qo:qo + ql],
                                      in_=ps2[qi][:ml, :ql], mul=COV_MUL)

            # ---- Stage 2: sim = rowsum((16d @ cov') * 16d)/2^14 + ssd ----
            for m in range(mt_count):
                for n in range(nw_count):
                    ps2 = [psum_s.tile([128, 392], f32, name="ps", tag="ps")
                           for _ in QT]
                    for j in range(NDR):
                        for qi, (qo, ql) in enumerate(QT):
                            nc.tensor.matmul(
                                ps2[qi][:, :ql],
                                lhsT=dtT[:, m, 2 * j:2 * j + 2, :],
                                rhs=cov8[:, n, 2 * j:2 * j + 2, qo:qo + ql],
                                start=(j == 0), stop=False, perf_mode=DR,
                            )
                    klast = KT[-1][1]
                    for qi, (qo, ql) in enumerate(QT):
                        nc.tensor.matmul(
                            ps2[qi][:, :ql], lhsT=dtT[:klast, m, NKT - 1, :],
                            rhs=cov8[:klast, n, NKT - 1, qo:qo + ql],
                            start=False, stop=True,
                        )
                    pp = stats.tile([128, 2], f32, tag="pp")
                    for qi, (qo, ql) in enumerate(QT):
                        scr8 = s8_pool.tile([128, 392], f8, tag="s8")
                        nc.vector.scalar_tensor_tensor(
                            out=scr8[:, :ql], in0=ps2[qi][:, :ql],
                            scalar=STT_MUL, in1=d_res[:, m, qo:qo + ql],
                            op0=OP.mult, op1=OP.mult,
                            accum_out=pp[:, qi:qi + 1],
                        )
                    red = stats.tile([128, 1], f32, tag="red")
                    nc.vector.tensor_reduce(
                        out=red, in_=pp, axis=mybir.AxisListType.X, op=OP.add
                    )
                    nc.vector.tensor_scalar(
                        out=out_acc[:, m, n:n + 1], in0=red,
                        scalar1=ssd[:, m:m + 1], scalar2=None, op0=OP.add,
                    )

            for m in range(mt_count):
                nc.sync.dma_start(
                    out=out[m * 128:(m + 1) * 128, :], in_=out_acc[:, m, :]
                )

    # Bacc defers register allocation to compile(); run_bass_via_pjrt
    # serializes the module as-is, so finalize here.
    nc.finalize()
    return nc


def get_program():
    key = "nc"
    if key not in _STATE:
        _STATE[key] = _build_program()
    return _STATE[key]


def make_in_maps(x1, x2):
    import ml_dtypes
    x1f = np.ascontiguousarray(
        np.asarray(x1, dtype=np.float32).reshape(B * C, HW)
    )
    x2q = np.ascontiguousarray(
        (np.asarray(x2, dtype=np.float32).reshape(NW, NR, HW) * XSC)
        .astype(ml_dtypes.float8_e4m3)
    )
    return [
        {"x1s": x1f[c * NI:(c + 1) * NI], "x2": x2q}
        for c in range(N_CORES)
    ]


def assemble_output(core_outs):
    # per-core (NI, NW) -> (BSH, NW*C); concat over cores -> (B, NW*C)
    parts = [
        o.reshape(BSH, C, NW).transpose(0, 2, 1).reshape(BSH, NW * C)
        for o in core_outs
    ]
    return np.ascontiguousarray(np.concatenate(parts, axis=0), dtype=np.float32)


def kernel(x1, x2):
    from concourse.bass_utils import run_bass_kernel_spmd

    nc = get_program()
    in_maps = make_in_maps(x1, x2)
    res = run_bass_kernel_spmd(nc, in_maps, list(range(N_CORES)))
    return assemble_output([res.results[i]["out"] for i in range(N_CORES)])
